# revision 17
# baseline (speedup 1.0000x reference)
"""Trainium2 Bass kernel for the AttentionTSSA module, 8-core SPMD.

Sharding: core c handles batch b = c // 2, token half = c % 2 (2048 tokens).
Everything is computed with features on SBUF partitions (w kept transposed as
wT[o, n]) so that per-feature statistics are free-dim reductions / per-partition
scalars.  The two cross-half reductions (sum_n w^2 and [sum_n pi, sum_n pi*w^2])
are exchanged with tiny pair-wise AllGathers (~4 KB).

Pipeline per core:
  GEMM1:  wT[o, n] = (x @ qkv_w.T).T          (PE, bf16, fp32 accum)
          wT2 = wT^2, ssq[o] = sum_n wT2      (fused DVE scalar_tensor_tensor)
  AG #1   -> full-n ssq;  scale[o] = temp[h(o)] / max(ssq, 1e-24)
  s[h,n]  = sum_o S[o,h] * wT2[o,n]           (PE matmul with selector S)
  pi      = softmax_h(s)  (exp on ACT, sum over h / broadcasts via tiny PE mms)
  pi_b    = E.T @ pi  (PE broadcast head -> 64 features)
  v       = wT * pi_b;  dots[o] = sum_n wT*v  (DVE, fused reduce)
  AG #2   -> full-n dots, pisum;  attn[o] = 1/(1 + dots/(pisum+1e-8))
  GEMM2:  out = v.T @ (-attn * out_w.T) + out_b
"""

import os
import sys

if "/opt/trn_rl_repo" not in sys.path:
    sys.path.insert(0, "/opt/trn_rl_repo")

import numpy as np
import ml_dtypes

import concourse.bass as bass
import concourse.bacc as bacc
import concourse.mybir as mybir
import concourse.tile as tile
from concourse.bass_utils import run_bass_kernel_spmd

def _ensure_ntff_hook():
    """Register the axon NTFF profiling hook if the antenv shim lacks it.

    Best-effort: grading runs with trace=False and never needs this."""
    import types

    try:
        from antenv.axon_hooks import get_axon_ntff_profile_hook  # noqa: F401

        return
    except ImportError:
        pass
    hook = None
    try:
        from trn_agent_boot.trn_boot import _ntff_profile_via_ctypes

        so = "/opt/axon/libaxon_pjrt.so"
        if os.path.exists(so):
            hook = _ntff_profile_via_ctypes(so)
    except Exception:
        hook = None
    m = types.ModuleType("antenv.axon_hooks")
    m.get_axon_ntff_profile_hook = lambda: hook
    m.set_axon_ntff_profile_hook = lambda h: None
    sys.modules["antenv.axon_hooks"] = m


_ensure_ntff_hook()

F32 = mybir.dt.float32
BF16 = mybir.dt.bfloat16
AF = mybir.ActivationFunctionType
ALU = mybir.AluOpType

B, N, C = 4, 4096, 1024
H, D = 16, 64
P = 128
NCORES = 8
NSH = N // 2            # tokens per core
KT = C // P             # 8 contraction tiles
OT = C // P             # 8 feature (output-of-gemm1) tiles
CHS = 512               # free-dim chunk size
NCH = NSH // CHS        # 4 chunks
RG = [[0, 1], [2, 3], [4, 5], [6, 7]]   # pairs sharing one batch

LAST_RESULTS = None     # populated by kernel() for test harness introspection


def _body(ctx, tc, xT, qkvT, owT, tempcol, outb, Ef, out):
    nc = tc.nc

    consts = ctx.enter_context(tc.tile_pool(name="consts", bufs=1))
    xpool = ctx.enter_context(tc.tile_pool(name="xpool", bufs=3))
    wpool = ctx.enter_context(tc.tile_pool(name="wpool", bufs=1))
    stat = ctx.enter_context(tc.tile_pool(name="stat", bufs=1))
    pibp = ctx.enter_context(tc.tile_pool(name="pibp", bufs=3))
    scrp = ctx.enter_context(tc.tile_pool(name="scrp", bufs=3))
    expp = ctx.enter_context(tc.tile_pool(name="expp", bufs=3))
    opool = ctx.enter_context(tc.tile_pool(name="opool", bufs=3))
    pmm = ctx.enter_context(tc.tile_pool(name="pmm", bufs=3, space="PSUM"))
    psm = ctx.enter_context(tc.tile_pool(name="psm", bufs=3, space="PSUM"))
    dram = ctx.enter_context(tc.tile_pool(name="dram", bufs=1, space="DRAM"))

    # ---- constants into SBUF ----
    qkv_sb = consts.tile([P, KT, C], BF16)
    nc.sync.dma_start(out=qkv_sb, in_=qkvT.rearrange("(k p) o -> p k o", p=P))
    ow_sb = consts.tile([P, KT, C], BF16)
    nc.sync.dma_start(out=ow_sb, in_=owT.rearrange("(k p) o -> p k o", p=P))
    tcol_sb = consts.tile([P, OT], F32)
    nc.sync.dma_start(out=tcol_sb, in_=tempcol[:, :])
    outb_sb = consts.tile([P, C], F32)
    nc.sync.dma_start(out=outb_sb, in_=outb[:, :].to_broadcast([P, C]))

    ones16_f = consts.tile([16, 1], F32)
    nc.vector.memset(ones16_f, 1.0)
    ones1_bf = consts.tile([1, 16], BF16)
    nc.vector.memset(ones1_bf, 1.0)

    # selector E[h, ot, j] = 1 when feature o = ot*128 + j belongs to head h
    # (host-provided: engine writes must start at partition 0/32/64/96)
    E32_sb = consts.tile([16, OT, P], F32)
    nc.sync.dma_start(out=E32_sb, in_=Ef.rearrange("h (t p) -> h t p", p=P))
    E_sb = consts.tile([16, OT, P], BF16)
    nc.vector.tensor_copy(E_sb, E32_sb)

    # ---- persistent big tensors ----
    wT_sb = wpool.tile([P, OT, NSH], BF16)    # w transposed: [o, n]
    wT2_sb = wpool.tile([P, OT, NSH], BF16)   # w^2
    v_sb = wpool.tile([P, OT, NSH], BF16)     # w * pi

    ssq_parts = stat.tile([P, OT, NCH], F32)

    # ---- phase 1: GEMM1 + w^2 + partial sum_n w^2 ----
    xT3 = xT.rearrange("(k p) n -> p k n", p=P)
    for ch in range(NCH):
        sl = slice(ch * CHS, (ch + 1) * CHS)
        x_t = xpool.tile([P, KT, CHS], BF16)
        nc.sync.dma_start(out=x_t, in_=xT3[:, :, sl])
        for ot in range(OT):
            ps = pmm.tile([P, CHS], F32)
            for k in range(KT):
                nc.tensor.matmul(
                    ps,
                    lhsT=qkv_sb[:, k, ot * P : (ot + 1) * P],
                    rhs=x_t[:, k, :],
                    start=(k == 0),
                    stop=(k == KT - 1),
                )
            wsl = wT_sb[:, ot, sl]
            nc.scalar.activation(out=wsl, in_=ps, func=AF.Copy)
            nc.vector.tensor_mul(wT2_sb[:, ot, sl], wsl, wsl)
            nc.vector.tensor_reduce(
                out=ssq_parts[:, ot, ch : ch + 1],
                in_=wT2_sb[:, ot, sl],
                axis=mybir.AxisListType.X,
                op=ALU.add,
            )

    ssq_l = stat.tile([P, OT], F32)
    nc.vector.tensor_reduce(
        out=ssq_l, in_=ssq_parts, axis=mybir.AxisListType.X, op=ALU.add
    )

    # ---- AG #1: exchange partial ssq with pair core ----
    cc1_in = dram.tile([P, OT], F32)
    cc1_out = dram.tile([2, P, OT], F32)
    nc.sync.dma_start(out=cc1_in, in_=ssq_l)
    nc.gpsimd.collective_compute(
        "AllGather",
        ALU.bypass,
        replica_groups=RG,
        ins=[cc1_in.opt()],
        outs=[cc1_out.opt()],
    )
    ssq_a = stat.tile([P, OT], F32)
    ssq_b = stat.tile([P, OT], F32)
    nc.sync.dma_start(out=ssq_a, in_=cc1_out[0])
    nc.sync.dma_start(out=ssq_b, in_=cc1_out[1])
    scale_col = stat.tile([P, OT], F32)
    nc.vector.tensor_add(scale_col, ssq_a, ssq_b)
    nc.vector.tensor_scalar_max(scale_col, scale_col, 1e-24)
    nc.vector.reciprocal(scale_col, scale_col)
    nc.vector.tensor_mul(scale_col, scale_col, tcol_sb)

    # selector S[j, ot, h] = scale[o] for h = h(o): s = S.T @ wT2
    S_sb = stat.tile([P, OT, 16], BF16)
    nc.vector.memset(S_sb, 0.0)
    for t in range(OT):
        nc.vector.tensor_copy(
            out=S_sb[0:64, t, 2 * t : 2 * t + 1], in_=scale_col[0:64, t : t + 1]
        )
        nc.vector.tensor_copy(
            out=S_sb[64:128, t, 2 * t + 1 : 2 * t + 2],
            in_=scale_col[64:128, t : t + 1],
        )

    # ---- phase 3: s, softmax over heads, pi ----
    zr_bf = stat.tile([1, NSH], BF16)
    pi_sb = stat.tile([16, NSH], BF16)
    pis_parts = stat.tile([16, NCH], F32)
    for ch in range(NCH):
        sl = slice(ch * CHS, (ch + 1) * CHS)
        s_ps = psm.tile([16, CHS], F32, tag="psmall")
        for k in range(KT):
            nc.tensor.matmul(
                s_ps,
                lhsT=S_sb[:, k, :],
                rhs=wT2_sb[:, k, sl],
                start=(k == 0),
                stop=(k == KT - 1),
            )
        exp_t = expp.tile([16, CHS], F32)
        nc.scalar.activation(out=exp_t, in_=s_ps, func=AF.Exp)
        z_ps = psm.tile([1, CHS], F32, tag="psmall")
        nc.tensor.matmul(z_ps, lhsT=ones16_f, rhs=exp_t, start=True, stop=True)
        with nc.allow_low_precision(reason="1/z broadcast in bf16 is intentional"):
            nc.vector.reciprocal(zr_bf[0:1, sl], z_ps)
        zb_ps = psm.tile([16, CHS], F32, tag="psmall")
        nc.tensor.matmul(
            zb_ps, lhsT=ones1_bf, rhs=zr_bf[0:1, sl], start=True, stop=True
        )
        nc.vector.tensor_mul(pi_sb[:, sl], exp_t, zb_ps)
        nc.vector.tensor_reduce(
            out=pis_parts[:, ch : ch + 1],
            in_=pi_sb[:, sl],
            axis=mybir.AxisListType.X,
            op=ALU.add,
        )

    pisum_l = stat.tile([16, 1], F32)
    nc.vector.tensor_reduce(
        out=pisum_l, in_=pis_parts, axis=mybir.AxisListType.X, op=ALU.add
    )

    # ---- phase 4: pi broadcast to features, v = w*pi, dots = sum_n w^2*pi ----
    dots_parts = stat.tile([P, OT, NCH], F32)
    for ch in range(NCH):
        sl = slice(ch * CHS, (ch + 1) * CHS)
        for ot in range(OT):
            pib_ps = psm.tile([P, CHS], F32, tag="psmall")
            nc.tensor.matmul(
                pib_ps, lhsT=E_sb[:, ot, :], rhs=pi_sb[:, sl], start=True, stop=True
            )
            pib_t = pibp.tile([P, CHS], BF16)
            nc.scalar.activation(out=pib_t, in_=pib_ps, func=AF.Copy)
            vsl = v_sb[:, ot, sl]
            nc.vector.tensor_mul(vsl, wT_sb[:, ot, sl], pib_t)
            scr_t = scrp.tile([P, CHS], BF16)
            nc.vector.tensor_mul(scr_t, wT_sb[:, ot, sl], vsl)
            nc.vector.tensor_reduce(
                out=dots_parts[:, ot, ch : ch + 1],
                in_=scr_t,
                axis=mybir.AxisListType.X,
                op=ALU.add,
            )

    dots_l = stat.tile([P, OT], F32)
    nc.vector.tensor_reduce(
        out=dots_l, in_=dots_parts, axis=mybir.AxisListType.X, op=ALU.add
    )

    # ---- AG #2: exchange partial dots + pisum ----
    pisum_pad = stat.tile([P, 1], F32)
    nc.vector.memset(pisum_pad, 0.0)
    nc.vector.tensor_copy(pisum_pad[0:16, :], pisum_l)
    cc2_in = dram.tile([P, OT + 1], F32)
    cc2_out = dram.tile([2, P, OT + 1], F32)
    nc.sync.dma_start(out=cc2_in[:, 0:OT], in_=dots_l)
    nc.sync.dma_start(out=cc2_in[:, OT : OT + 1], in_=pisum_pad)
    nc.gpsimd.collective_compute(
        "AllGather",
        ALU.bypass,
        replica_groups=RG,
        ins=[cc2_in.opt()],
        outs=[cc2_out.opt()],
    )
    st_a = stat.tile([P, OT + 1], F32)
    st_b = stat.tile([P, OT + 1], F32)
    nc.sync.dma_start(out=st_a, in_=cc2_out[0])
    nc.sync.dma_start(out=st_b, in_=cc2_out[1])
    st_f = stat.tile([P, OT + 1], F32)
    nc.vector.tensor_add(st_f, st_a, st_b)

    # attn[o] = 1 / (1 + dots[o] / (pisum[h(o)] + 1e-8)); use -attn
    rpi = stat.tile([16, 1], F32)
    nc.vector.tensor_scalar_add(rpi, st_f[0:16, OT : OT + 1], 1e-8)
    nc.vector.reciprocal(rpi, rpi)
    rep_sb = stat.tile([P, OT], F32)
    for t in range(OT):
        rep_ps = psm.tile([P, 1], F32, tag="psmall")
        nc.tensor.matmul(rep_ps, lhsT=E32_sb[:, t, :], rhs=rpi, start=True, stop=True)
        nc.scalar.activation(out=rep_sb[:, t : t + 1], in_=rep_ps, func=AF.Copy)
    natt = stat.tile([P, OT], F32)
    nc.vector.tensor_mul(natt, st_f[:, 0:OT], rep_sb)
    nc.vector.tensor_scalar_add(natt, natt, 1.0)
    nc.vector.reciprocal(natt, natt)
    nc.vector.tensor_scalar_mul(natt, natt, -1.0)

    # fold -attn into out_w.T rows (per-partition scale on ACT)
    for k in range(KT):
        nc.scalar.activation(
            out=ow_sb[:, k, :],
            in_=ow_sb[:, k, :],
            func=AF.Copy,
            scale=natt[:, k : k + 1],
        )

    # ---- phase 6: GEMM2 + bias ----
    for nt in range(NSH // P):
        nsl = slice(nt * P, (nt + 1) * P)
        for oc in range(C // CHS):
            osl = slice(oc * CHS, (oc + 1) * CHS)
            ps = pmm.tile([P, CHS], F32)
            for k in range(KT):
                nc.tensor.matmul(
                    ps,
                    lhsT=v_sb[:, k, nsl],
                    rhs=ow_sb[:, k, osl],
                    start=(k == 0),
                    stop=(k == KT - 1),
                )
            o_t = opool.tile([P, CHS], F32)
            nc.vector.tensor_add(o_t, ps, outb_sb[:, osl])
            nc.sync.dma_start(out=out[nsl, osl], in_=o_t)


def build_nc():
    nc = bacc.Bacc("TRN2", target_bir_lowering=False, num_devices=NCORES)
    xT = nc.dram_tensor("xT", [C, NSH], BF16, kind="ExternalInput")
    qkvT = nc.dram_tensor("qkvT", [C, C], BF16, kind="ExternalInput")
    owT = nc.dram_tensor("owT", [C, C], BF16, kind="ExternalInput")
    tempcol = nc.dram_tensor("tempcol", [P, OT], F32, kind="ExternalInput")
    outb = nc.dram_tensor("outb", [1, C], F32, kind="ExternalInput")
    Ef = nc.dram_tensor("Ef", [16, C], F32, kind="ExternalInput")
    out = nc.dram_tensor("out", [NSH, C], F32, kind="ExternalOutput")

    from contextlib import ExitStack

    with tile.TileContext(nc) as tc, ExitStack() as ctx:
        _body(ctx, tc, xT, qkvT, owT, tempcol, outb, Ef, out)
    nc.finalize()
    return nc


def make_in_maps(x, qkv_w, temp, out_w, out_b):
    bf = ml_dtypes.bfloat16
    qkvT = np.ascontiguousarray(qkv_w.T).astype(bf)
    owT = np.ascontiguousarray(out_w.T).astype(bf)
    o_idx = np.arange(OT)[None, :] * P + np.arange(P)[:, None]   # [P, OT]
    tempcol = np.ascontiguousarray(
        temp.reshape(H)[o_idx // D].astype(np.float32)
    )
    outb = np.ascontiguousarray(out_b.reshape(1, C).astype(np.float32))
    o_all = np.arange(C)
    Ef = np.ascontiguousarray(
        (np.arange(H)[:, None] == (o_all[None, :] // D)).astype(np.float32)
    )

    in_maps = []
    for core in range(NCORES):
        b, half = core // 2, core % 2
        xs = x[b, half * NSH : (half + 1) * NSH, :]
        xT = np.ascontiguousarray(xs.T).astype(bf)
        in_maps.append(
            {
                "xT": xT,
                "qkvT": qkvT,
                "owT": owT,
                "tempcol": tempcol,
                "outb": outb,
                "Ef": Ef,
            }
        )
    return in_maps


def assemble_out(results):
    out = np.empty((B, N, C), np.float32)
    for core in range(NCORES):
        b, half = core // 2, core % 2
        out[b, half * NSH : (half + 1) * NSH, :] = results[core]["out"]
    return out


def kernel(**inputs):
    global LAST_RESULTS
    x = np.asarray(inputs["x"], np.float32)
    qkv_w = np.asarray(inputs["qkv_w"], np.float32)
    temp = np.asarray(inputs["temp"], np.float32)
    out_w = np.asarray(inputs["out_w"], np.float32)
    out_b = np.asarray(inputs["out_b"], np.float32)

    in_maps = make_in_maps(x, qkv_w, temp, out_w, out_b)
    nc = build_nc()
    res = run_bass_kernel_spmd(
        nc,
        in_maps,
        core_ids=list(range(NCORES)),
        trace=bool(os.environ.get("BASS_TRACE_KERNEL")),
    )
    LAST_RESULTS = res
    if res.exec_time_ns is not None:
        print(f"HW exec time: {res.exec_time_ns} ns")
    return assemble_out(res.results)


if __name__ == "__main__":
    nc = build_nc()
    print("built ok")


# revision 21
# speedup vs baseline: 1.0649x; 1.0649x over previous
"""Trainium2 Bass kernel for the AttentionTSSA module, 8-core SPMD.

Sharding: core c handles batch b = c // 2, token half = c % 2 (2048 tokens).
Everything is computed with features on SBUF partitions (w kept transposed as
wT[o, n]) so that per-feature statistics are free-dim reductions / per-partition
scalars.  The two cross-half reductions (sum_n w^2 and [sum_n pi, sum_n pi*w^2])
are exchanged with tiny pair-wise AllGathers (~4 KB).

Pipeline per core:
  GEMM1:  wT[o, n] = (x @ qkv_w.T).T          (PE, bf16, fp32 accum)
          wT2 = wT^2, ssq[o] = sum_n wT2      (DVE mul + free-dim reduce)
  AG #1   -> full-n ssq;  scale[o] = temp[h(o)] / max(ssq, 1e-24)
  s[h,n]  = sum_o S[o,h] * wT2[o,n]           (PE matmul with selector S)
  pi      = softmax_h(s)  (exp on ACT, sum over h / broadcasts via tiny PE mms)
  pi_b    = E.T @ pi  (PE broadcast head -> 64 features)
  v       = wT * pi_b;  dots[o] = sum_n wT*v  (DVE, fused reduce)
  AG #2   -> full-n dots, pisum;  attn[o] = 1/(1 + dots/(pisum+1e-8))
  GEMM2:  out = v.T @ (-attn * out_w.T) + out_b
"""

import os
import sys

if "/opt/trn_rl_repo" not in sys.path:
    sys.path.insert(0, "/opt/trn_rl_repo")

import numpy as np
import ml_dtypes

import concourse.bass as bass
import concourse.bacc as bacc
import concourse.mybir as mybir
import concourse.tile as tile
from concourse.bass_utils import run_bass_kernel_spmd

def _ensure_ntff_hook():
    """Register the axon NTFF profiling hook if the antenv shim lacks it.

    Best-effort: grading runs with trace=False and never needs this."""
    import types

    try:
        from antenv.axon_hooks import get_axon_ntff_profile_hook  # noqa: F401

        return
    except ImportError:
        pass
    hook = None
    try:
        from trn_agent_boot.trn_boot import _ntff_profile_via_ctypes

        so = "/opt/axon/libaxon_pjrt.so"
        if os.path.exists(so):
            hook = _ntff_profile_via_ctypes(so)
    except Exception:
        hook = None
    m = types.ModuleType("antenv.axon_hooks")
    m.get_axon_ntff_profile_hook = lambda: hook
    m.set_axon_ntff_profile_hook = lambda h: None
    sys.modules["antenv.axon_hooks"] = m


_ensure_ntff_hook()

F32 = mybir.dt.float32
BF16 = mybir.dt.bfloat16
AF = mybir.ActivationFunctionType
ALU = mybir.AluOpType

B, N, C = 4, 4096, 1024
H, D = 16, 64
P = 128
NCORES = 8
NSH = N // 2            # tokens per core
KT = C // P             # 8 contraction tiles
OT = C // P             # 8 feature (output-of-gemm1) tiles
CHS = 512               # free-dim chunk size
NCH = NSH // CHS        # 4 chunks
RG = [[0, 1], [2, 3], [4, 5], [6, 7]]   # pairs sharing one batch

LAST_RESULTS = None     # populated by kernel() for test harness introspection


def _body(ctx, tc, xT, qkvT, owT, tempcol, outb, Ef, out):
    nc = tc.nc

    consts = ctx.enter_context(tc.tile_pool(name="consts", bufs=1))
    xpool = ctx.enter_context(tc.tile_pool(name="xpool", bufs=3))
    wpool = ctx.enter_context(tc.tile_pool(name="wpool", bufs=1))
    stat = ctx.enter_context(tc.tile_pool(name="stat", bufs=1))
    pibp = ctx.enter_context(tc.tile_pool(name="pibp", bufs=3))
    scrp = ctx.enter_context(tc.tile_pool(name="scrp", bufs=2))
    expp = ctx.enter_context(tc.tile_pool(name="expp", bufs=3))
    opool = ctx.enter_context(tc.tile_pool(name="opool", bufs=3))
    pmm = ctx.enter_context(tc.tile_pool(name="pmm", bufs=3, space="PSUM"))
    psm = ctx.enter_context(tc.tile_pool(name="psm", bufs=3, space="PSUM"))
    dram = ctx.enter_context(tc.tile_pool(name="dram", bufs=1, space="DRAM"))

    # ---- constants into SBUF ----
    qkv_sb = consts.tile([P, KT, C], BF16)
    nc.sync.dma_start(out=qkv_sb, in_=qkvT.rearrange("(k p) o -> p k o", p=P))
    ow_sb = consts.tile([P, KT, C], BF16)
    nc.sync.dma_start(out=ow_sb, in_=owT.rearrange("(k p) o -> p k o", p=P))
    tcol_sb = consts.tile([P, OT], F32)
    nc.sync.dma_start(out=tcol_sb, in_=tempcol[:, :])
    outb_sb = consts.tile([P, C], F32)
    nc.sync.dma_start(out=outb_sb, in_=outb[:, :].to_broadcast([P, C]))

    ones16_f = consts.tile([16, 1], F32)
    nc.vector.memset(ones16_f, 1.0)
    ones1_bf = consts.tile([1, 16], BF16)
    nc.vector.memset(ones1_bf, 1.0)

    # selector E[h, ot, j] = 1 when feature o = ot*128 + j belongs to head h
    # (host-provided: engine writes must start at partition 0/32/64/96)
    E32_sb = consts.tile([16, OT, P], F32)
    nc.sync.dma_start(out=E32_sb, in_=Ef.rearrange("h (t p) -> h t p", p=P))
    E_sb = consts.tile([16, OT, P], BF16)
    nc.vector.tensor_copy(E_sb, E32_sb)

    # ---- persistent big tensors ----
    wT_sb = wpool.tile([P, OT, NSH], BF16)    # w transposed: [o, n]
    wT2_sb = wpool.tile([P, OT, NSH], BF16)   # w^2
    v_sb = wpool.tile([P, OT, NSH], BF16)     # w * pi

    ssq_parts = stat.tile([P, OT, NCH], F32)

    # ---- phase 1: GEMM1 + w^2 + partial sum_n w^2 ----
    xT3 = xT.rearrange("(k p) n -> p k n", p=P)
    for ch in range(NCH):
        sl = slice(ch * CHS, (ch + 1) * CHS)
        x_t = xpool.tile([P, KT, CHS], BF16)
        nc.sync.dma_start(out=x_t, in_=xT3[:, :, sl])
        for ot in range(OT):
            ps = pmm.tile([P, CHS], F32)
            for k in range(KT):
                nc.tensor.matmul(
                    ps,
                    lhsT=qkv_sb[:, k, ot * P : (ot + 1) * P],
                    rhs=x_t[:, k, :],
                    start=(k == 0),
                    stop=(k == KT - 1),
                )
            wsl = wT_sb[:, ot, sl]
            nc.scalar.activation(out=wsl, in_=ps, func=AF.Copy)
            nc.vector.tensor_mul(wT2_sb[:, ot, sl], wsl, wsl)

    # one row-granularity reduce per feature tile (8 big instrs, not 64 small)
    ssq_l = stat.tile([P, OT], F32)
    for ot in range(OT):
        nc.vector.tensor_reduce(
            out=ssq_l[:, ot : ot + 1],
            in_=wT2_sb[:, ot, :],
            axis=mybir.AxisListType.X,
            op=ALU.add,
        )

    # ---- AG #1: exchange partial ssq with pair core ----
    cc1_in = dram.tile([P, OT], F32)
    cc1_out = dram.tile([2, P, OT], F32)
    nc.sync.dma_start(out=cc1_in, in_=ssq_l)
    nc.gpsimd.collective_compute(
        "AllGather",
        ALU.bypass,
        replica_groups=RG,
        ins=[cc1_in.opt()],
        outs=[cc1_out.opt()],
    )
    ssq_a = stat.tile([P, OT], F32)
    ssq_b = stat.tile([P, OT], F32)
    nc.sync.dma_start(out=ssq_a, in_=cc1_out[0])
    nc.sync.dma_start(out=ssq_b, in_=cc1_out[1])
    scale_col = stat.tile([P, OT], F32)
    nc.vector.tensor_add(scale_col, ssq_a, ssq_b)
    nc.vector.tensor_scalar_max(scale_col, scale_col, 1e-24)
    nc.vector.reciprocal(scale_col, scale_col)
    nc.vector.tensor_mul(scale_col, scale_col, tcol_sb)

    # selector S[j, ot, h] = scale[o] for h = h(o): s = S.T @ wT2
    S_sb = stat.tile([P, OT, 16], BF16)
    nc.vector.memset(S_sb, 0.0)
    for t in range(OT):
        nc.vector.tensor_copy(
            out=S_sb[0:64, t, 2 * t : 2 * t + 1], in_=scale_col[0:64, t : t + 1]
        )
        nc.vector.tensor_copy(
            out=S_sb[64:128, t, 2 * t + 1 : 2 * t + 2],
            in_=scale_col[64:128, t : t + 1],
        )

    # ---- phase 3: s, softmax over heads, pi ----
    zr_bf = stat.tile([1, NSH], BF16)
    pi_sb = stat.tile([16, NSH], BF16)
    pis_parts = stat.tile([16, NCH], F32)
    for ch in range(NCH):
        sl = slice(ch * CHS, (ch + 1) * CHS)
        s_ps = psm.tile([16, CHS], F32, tag="psmall")
        for k in range(KT):
            nc.tensor.matmul(
                s_ps,
                lhsT=S_sb[:, k, :],
                rhs=wT2_sb[:, k, sl],
                start=(k == 0),
                stop=(k == KT - 1),
            )
        exp_t = expp.tile([16, CHS], F32)
        nc.scalar.activation(out=exp_t, in_=s_ps, func=AF.Exp)
        z_ps = psm.tile([1, CHS], F32, tag="psmall")
        nc.tensor.matmul(z_ps, lhsT=ones16_f, rhs=exp_t, start=True, stop=True)
        with nc.allow_low_precision(reason="1/z broadcast in bf16 is intentional"):
            nc.vector.reciprocal(zr_bf[0:1, sl], z_ps)
        zb_ps = psm.tile([16, CHS], F32, tag="psmall")
        nc.tensor.matmul(
            zb_ps, lhsT=ones1_bf, rhs=zr_bf[0:1, sl], start=True, stop=True
        )
        nc.vector.tensor_mul(pi_sb[:, sl], exp_t, zb_ps)
        nc.vector.tensor_reduce(
            out=pis_parts[:, ch : ch + 1],
            in_=pi_sb[:, sl],
            axis=mybir.AxisListType.X,
            op=ALU.add,
        )

    pisum_l = stat.tile([16, 1], F32)
    nc.vector.tensor_reduce(
        out=pisum_l, in_=pis_parts, axis=mybir.AxisListType.X, op=ALU.add
    )

    # ---- phase 4: pi broadcast to features, v = w*pi, dots = sum_n w^2*pi ----
    dots_l = stat.tile([P, OT], F32)
    for ot in range(OT):
        scr_row = scrp.tile([P, NSH], BF16)
        for ch in range(NCH):
            sl = slice(ch * CHS, (ch + 1) * CHS)
            pib_ps = psm.tile([P, CHS], F32, tag="psmall")
            nc.tensor.matmul(
                pib_ps, lhsT=E_sb[:, ot, :], rhs=pi_sb[:, sl], start=True, stop=True
            )
            pib_t = pibp.tile([P, CHS], BF16)
            nc.scalar.activation(out=pib_t, in_=pib_ps, func=AF.Copy)
            vsl = v_sb[:, ot, sl]
            nc.vector.tensor_mul(vsl, wT_sb[:, ot, sl], pib_t)
            nc.vector.tensor_mul(scr_row[:, sl], wT_sb[:, ot, sl], vsl)
        nc.vector.tensor_reduce(
            out=dots_l[:, ot : ot + 1],
            in_=scr_row,
            axis=mybir.AxisListType.X,
            op=ALU.add,
        )

    # ---- AG #2: exchange partial dots + pisum ----
    pisum_pad = stat.tile([P, 1], F32)
    nc.vector.memset(pisum_pad, 0.0)
    nc.vector.tensor_copy(pisum_pad[0:16, :], pisum_l)
    cc2_in = dram.tile([P, OT + 1], F32)
    cc2_out = dram.tile([2, P, OT + 1], F32)
    nc.sync.dma_start(out=cc2_in[:, 0:OT], in_=dots_l)
    nc.sync.dma_start(out=cc2_in[:, OT : OT + 1], in_=pisum_pad)
    nc.gpsimd.collective_compute(
        "AllGather",
        ALU.bypass,
        replica_groups=RG,
        ins=[cc2_in.opt()],
        outs=[cc2_out.opt()],
    )
    st_a = stat.tile([P, OT + 1], F32)
    st_b = stat.tile([P, OT + 1], F32)
    nc.sync.dma_start(out=st_a, in_=cc2_out[0])
    nc.sync.dma_start(out=st_b, in_=cc2_out[1])
    st_f = stat.tile([P, OT + 1], F32)
    nc.vector.tensor_add(st_f, st_a, st_b)

    # attn[o] = 1 / (1 + dots[o] / (pisum[h(o)] + 1e-8)); use -attn
    rpi = stat.tile([16, 1], F32)
    nc.vector.tensor_scalar_add(rpi, st_f[0:16, OT : OT + 1], 1e-8)
    nc.vector.reciprocal(rpi, rpi)
    rep_sb = stat.tile([P, OT], F32)
    for t in range(OT):
        rep_ps = psm.tile([P, 1], F32, tag="psmall")
        nc.tensor.matmul(rep_ps, lhsT=E32_sb[:, t, :], rhs=rpi, start=True, stop=True)
        nc.scalar.activation(out=rep_sb[:, t : t + 1], in_=rep_ps, func=AF.Copy)
    natt = stat.tile([P, OT], F32)
    nc.vector.tensor_mul(natt, st_f[:, 0:OT], rep_sb)
    nc.vector.tensor_scalar_add(natt, natt, 1.0)
    nc.vector.reciprocal(natt, natt)
    nc.vector.tensor_scalar_mul(natt, natt, -1.0)

    # fold -attn into out_w.T rows (per-partition scale on ACT)
    for k in range(KT):
        nc.scalar.activation(
            out=ow_sb[:, k, :],
            in_=ow_sb[:, k, :],
            func=AF.Copy,
            scale=natt[:, k : k + 1],
        )

    # ---- phase 6: GEMM2 + bias ----
    for nt in range(NSH // P):
        nsl = slice(nt * P, (nt + 1) * P)
        for oc in range(C // CHS):
            osl = slice(oc * CHS, (oc + 1) * CHS)
            ps = pmm.tile([P, CHS], F32)
            for k in range(KT):
                nc.tensor.matmul(
                    ps,
                    lhsT=v_sb[:, k, nsl],
                    rhs=ow_sb[:, k, osl],
                    start=(k == 0),
                    stop=(k == KT - 1),
                )
            o_t = opool.tile([P, CHS], F32)
            nc.vector.tensor_add(o_t, ps, outb_sb[:, osl])
            nc.sync.dma_start(out=out[nsl, osl], in_=o_t)


def build_nc():
    nc = bacc.Bacc("TRN2", target_bir_lowering=False, num_devices=NCORES)
    xT = nc.dram_tensor("xT", [C, NSH], BF16, kind="ExternalInput")
    qkvT = nc.dram_tensor("qkvT", [C, C], BF16, kind="ExternalInput")
    owT = nc.dram_tensor("owT", [C, C], BF16, kind="ExternalInput")
    tempcol = nc.dram_tensor("tempcol", [P, OT], F32, kind="ExternalInput")
    outb = nc.dram_tensor("outb", [1, C], F32, kind="ExternalInput")
    Ef = nc.dram_tensor("Ef", [16, C], F32, kind="ExternalInput")
    out = nc.dram_tensor("out", [NSH, C], F32, kind="ExternalOutput")

    from contextlib import ExitStack

    with tile.TileContext(nc) as tc, ExitStack() as ctx:
        _body(ctx, tc, xT, qkvT, owT, tempcol, outb, Ef, out)
    nc.finalize()
    return nc


def make_in_maps(x, qkv_w, temp, out_w, out_b):
    bf = ml_dtypes.bfloat16
    qkvT = np.ascontiguousarray(qkv_w.T).astype(bf)
    owT = np.ascontiguousarray(out_w.T).astype(bf)
    o_idx = np.arange(OT)[None, :] * P + np.arange(P)[:, None]   # [P, OT]
    tempcol = np.ascontiguousarray(
        temp.reshape(H)[o_idx // D].astype(np.float32)
    )
    outb = np.ascontiguousarray(out_b.reshape(1, C).astype(np.float32))
    o_all = np.arange(C)
    Ef = np.ascontiguousarray(
        (np.arange(H)[:, None] == (o_all[None, :] // D)).astype(np.float32)
    )

    in_maps = []
    for core in range(NCORES):
        b, half = core // 2, core % 2
        xs = x[b, half * NSH : (half + 1) * NSH, :]
        xT = np.ascontiguousarray(xs.T).astype(bf)
        in_maps.append(
            {
                "xT": xT,
                "qkvT": qkvT,
                "owT": owT,
                "tempcol": tempcol,
                "outb": outb,
                "Ef": Ef,
            }
        )
    return in_maps


def assemble_out(results):
    out = np.empty((B, N, C), np.float32)
    for core in range(NCORES):
        b, half = core // 2, core % 2
        out[b, half * NSH : (half + 1) * NSH, :] = results[core]["out"]
    return out


def kernel(**inputs):
    global LAST_RESULTS
    x = np.asarray(inputs["x"], np.float32)
    qkv_w = np.asarray(inputs["qkv_w"], np.float32)
    temp = np.asarray(inputs["temp"], np.float32)
    out_w = np.asarray(inputs["out_w"], np.float32)
    out_b = np.asarray(inputs["out_b"], np.float32)

    in_maps = make_in_maps(x, qkv_w, temp, out_w, out_b)
    nc = build_nc()
    res = run_bass_kernel_spmd(
        nc,
        in_maps,
        core_ids=list(range(NCORES)),
        trace=bool(os.environ.get("BASS_TRACE_KERNEL")),
    )
    LAST_RESULTS = res
    if res.exec_time_ns is not None:
        print(f"HW exec time: {res.exec_time_ns} ns")
    return assemble_out(res.results)


if __name__ == "__main__":
    nc = build_nc()
    print("built ok")
